# revision 15
# baseline (speedup 1.0000x reference)
"""GPT self-attention layer (B=2, S=2048, D=1024, H=16, hd=64) on 8 TRN2 cores.

Sharding: data-parallel over batch (2) x tensor-parallel over heads (4 groups
of 4 heads). Core c handles batch b=c//4, head group g=c%4.

v3 (fp16, rden-in-A2A):
  - All PE operands fp16 (host-converted): fast weight load, no fp32r
    penalties, half DMA + collective bytes.
  - Host math folds: bk dropped (softmax shift invariance), bv folded into
    bo (bo2 = bv@Wo + bo), 1/sqrt(hd) folded into Wq/bq.
  - ACT runs ONLY Exp (single table load); Q/K psum evacuation on DVE;
    causal masks and receive-side normalize run on GpSimd so the DVE queue
    never blocks attention staging behind normalization work.
  - Softmax denominators: the ctx matmul's interleaved ones-column
    accumulates den into psum row 64.  Each (pair, qt) DMAs its den rows
    into den_acc; after a pair's last q-block ONE [8, 512] DVE reciprocal
    produces rden for the whole pair (a [1, 512] reciprocal costs 8 cyc/elem
    -- batching partitions is the only way it's cheap).  rden rows are
    staged into the A2A slots (130 rows per shard: 2 heads x (64 ctx + 1
    rden)).  The receiver runs ONE 1024-index gather per pair pulling ctx
    and broadcast rden together, then one gpsimd multiply normalizes.
  - A tiny warm-up AllToAll at kernel start absorbs CC stream setup.
"""

import contextlib
import ctypes
import sys
import types

sys.path.insert(0, "/opt/trn_rl_repo")

import numpy as np

import concourse.bass as bass
import concourse.mybir as mybir
import concourse.tile as tile
from concourse import bacc
from concourse import bass_utils

P = 128
B, S, D = 2, 2048, 1024
NH_LOC = 4          # heads per core
HD = 64             # head dim
G = NH_LOC * HD     # local head dims = 256
MC = G // P         # m-chunks of local dims = 2 (== head pairs)
DC = D // P         # d-chunks = 8
TB = 512            # token block (output tokens per core, q-tile width)
NQT = S // TB       # q-tiles = 4
NTC = S // P        # token chunks = 16
NC = 8
SR = 2 * (HD + 1)   # A2A slot rows per shard = 130

F32 = mybir.dt.float32
F16 = mybir.dt.float16
I16 = mybir.dt.int16
Exp = mybir.ActivationFunctionType.Exp
MULT = mybir.AluOpType.mult
ADD = mybir.AluOpType.add


def _install_ntff_hook():
    """Make trace=True work under axon: inject antenv.axon_hooks backed by
    ctypes calls into libaxon_pjrt.so (mirrors trn_agent_boot logic)."""
    if "antenv.axon_hooks" in sys.modules:
        return
    holder = {}
    mod = types.ModuleType("antenv.axon_hooks")
    mod.set_axon_ntff_profile_hook = lambda h: holder.update(h=h)
    mod.get_axon_ntff_profile_hook = lambda: holder.get("h")
    sys.modules["antenv.axon_hooks"] = mod
    try:
        lib = ctypes.CDLL("/opt/axon/libaxon_pjrt.so")
        if not hasattr(lib, "axon_start_nrt_profile"):
            return
    except OSError:
        return
    lib.axon_start_nrt_profile.argtypes = [
        ctypes.POINTER(ctypes.c_int64),
        ctypes.c_size_t,
    ]
    lib.axon_start_nrt_profile.restype = ctypes.c_int64
    lib.axon_stop_nrt_profile.argtypes = [ctypes.c_char_p]
    lib.axon_stop_nrt_profile.restype = ctypes.c_int64

    @contextlib.contextmanager
    def _hook(output_dir, device_ids):
        import jax

        jax.devices()
        if device_ids:
            ids = (ctypes.c_int64 * len(device_ids))(*device_ids)
            rc = lib.axon_start_nrt_profile(ids, len(device_ids))
        else:
            rc = lib.axon_start_nrt_profile(None, 0)
        if rc != 0:
            raise RuntimeError(f"axon_start_nrt_profile rc={rc}")
        try:
            yield
        finally:
            n = lib.axon_stop_nrt_profile(str(output_dir).encode())
            print(f"profile: {n} ntff file(s) written to {output_dir}")

    holder["h"] = _hook


def build(coll=True):
    nc = bacc.Bacc("TRN2", target_bir_lowering=False, debug=False, num_devices=NC)

    xt_d = nc.dram_tensor("xt", [D, S], F16, kind="ExternalInput").ap()
    wq_d = nc.dram_tensor("wq", [D, G], F16, kind="ExternalInput").ap()
    wk_d = nc.dram_tensor("wk", [D, G], F16, kind="ExternalInput").ap()
    wv_d = nc.dram_tensor("wv", [D, G], F16, kind="ExternalInput").ap()
    bq_d = nc.dram_tensor("bq", [P, MC], F32, kind="ExternalInput").ap()
    wo_d = nc.dram_tensor("wo", [D, D], F16, kind="ExternalInput").ap()
    bo2_d = nc.dram_tensor("bo2", [1, D], F32, kind="ExternalInput").ap()
    gidx_d = nc.dram_tensor("gidx", [P, D // 16], I16, kind="ExternalInput").ap()
    y_d = nc.dram_tensor("y", [TB, D], F32, kind="ExternalOutput").ap()

    with tile.TileContext(nc) as tc:
        with (
            tc.tile_pool(name="const", bufs=1) as const,
            tc.tile_pool(name="dram", bufs=1, space="DRAM") as dram,
            tc.tile_pool(name="ps", bufs=3, space="PSUM") as ps,
            tc.tile_pool(name="ps_ctx", bufs=1, space="PSUM") as ps_ctx,
            tc.tile_pool(name="persist", bufs=1) as persist,
            tc.tile_pool(name="pTp", bufs=8) as pTp,
            tc.tile_pool(name="cxp", bufs=4) as cxp,
        ):
            # warm-up collective: absorbs CC stream setup + initial barrier
            dummy_in = dram.tile([NC, 8], F16, tag="dummy_in")
            dummy_out = dram.tile([NC, 8], F16, tag="dummy_out")
            if coll:
                nc.gpsimd.collective_compute(
                    "AllToAll",
                    mybir.AluOpType.bypass,
                    ins=[dummy_in.opt()],
                    outs=[dummy_out.opt()],
                    replica_groups=[list(range(NC))],
                )

            # ---------------- weights/constants (DMA order matters) -------
            wq_sb = persist.tile([P, DC, G], F16, tag="wq")
            wk_sb = persist.tile([P, DC, G], F16, tag="wk")
            wv_sb = persist.tile([P, DC, G], F16, tag="wv")
            xTg = [
                persist.tile([P, DC, TB], F16, tag=f"xT{g}", name=f"xT{g}")
                for g in range(NQT)
            ]
            xt_r = xt_d.rearrange("(dc p) t -> p dc t", p=P)
            wq_r = wq_d.rearrange("(dc p) m -> p dc m", p=P)
            # interleave per-dc chunks so the first matmul starts early
            for dc in range(DC):
                nc.sync.dma_start(wq_sb[:, dc], wq_r[:, dc])
                nc.sync.dma_start(xTg[0][:, dc], xt_r[:, dc, 0:TB])
            bq_sb = const.tile([P, MC], F32, tag="bq")
            nc.sync.dma_start(bq_sb[:], bq_d)

            # trimask[k, u] = 1 if k <= u else 0 (keep where u - k >= 0)
            tri_f = const.tile([P, P], F32, tag="tri_f")
            nc.gpsimd.memset(tri_f[:], 1.0)
            nc.gpsimd.affine_select(
                out=tri_f[:],
                in_=tri_f[:],
                compare_op=mybir.AluOpType.is_ge,
                fill=0.0,
                base=0,
                pattern=[[1, P]],
                channel_multiplier=-1,
            )
            tri_h = const.tile([P, P], F16, tag="tri_h")
            nc.vector.tensor_copy(tri_h[:], tri_f[:])

            # persistent activations
            qT = persist.tile([P, MC, S], F16, tag="qT")
            kT = persist.tile([P, MC, S], F16, tag="kT")
            v_sb = persist.tile([P, NTC, NH_LOC * (HD + 1)], F16, tag="v")
            wo_sb = persist.tile([P, DC, D], F16, tag="wo")

            # ones columns of v (denominator trick): col 64 of each head block
            v_heads = v_sb[:].rearrange("p t (h c) -> p t h c", c=HD + 1)
            nc.vector.memset(v_heads[:, :, :, HD], 1.0)

            a2a_in = [
                dram.tile([NC * SR, TB], F16, name=f"a2ain{p}", tag=f"a2ain{p}")
                for p in range(2)
            ]
            a2a_out = [
                dram.tile([NC * SR, TB], F16, name=f"a2aout{p}", tag=f"a2aout{p}")
                for p in range(2)
            ]
            den_acc = [
                persist.tile([2 * NQT, TB], F16, tag=f"dacc{p}", name=f"dacc{p}")
                for p in range(2)
            ]

            def proj_group(g):
                """QT/KT/V projections for 512-token group g."""
                if g == 0:
                    nc.sync.dma_start(
                        wk_sb[:], wk_d.rearrange("(dc p) m -> p dc m", p=P)
                    )
                    nc.sync.dma_start(
                        wv_sb[:], wv_d.rearrange("(dc p) m -> p dc m", p=P)
                    )
                for w_sb, has_b, out_t in ((wq_sb, True, qT), (wk_sb, False, kT)):
                    for mc_i in range(MC):
                        pj = ps.tile([P, 2, 512], F32, tag="mm")
                        for dc in range(DC):
                            nc.tensor.matmul(
                                pj[:, 0, :],
                                w_sb[:, dc, mc_i * P : (mc_i + 1) * P],
                                xTg[g][:, dc, :],
                                start=(dc == 0),
                                stop=(dc == DC - 1),
                            )
                        dst = out_t[:, mc_i, g * TB : (g + 1) * TB]
                        if has_b:
                            nc.vector.tensor_scalar_add(
                                dst, pj[:, 0, :], bq_sb[:, mc_i : mc_i + 1]
                            )
                        else:
                            nc.vector.tensor_copy(dst, pj[:, 0, :])
                for ti in range(4):
                    tc_i = 4 * g + ti
                    pv = ps.tile([P, 2, 512], F32, tag="mm")
                    for dc in range(DC):
                        nc.tensor.matmul(
                            pv[:, 0, 0:G],
                            xTg[g][:, dc, ti * P : (ti + 1) * P],
                            wv_sb[:, dc, :],
                            start=(dc == 0),
                            stop=(dc == DC - 1),
                        )
                    nc.vector.tensor_copy(
                        v_heads[:, tc_i, :, 0:HD],
                        pv[:, 0, 0:G].rearrange("p (h c) -> p h c", c=HD),
                    )

            def attn(pair, qt):
                """Attention for head pair `pair`, q-block `qt`; stages raw
                ctx rows into a2a_in[pair] and den rows into den_acc."""
                nkc = 4 * qt + 4
                c_ps = ps_ctx.tile([P, 2, 512], F32, tag="ctx")
                for kc in range(nkc):
                    j = kc - 4 * qt
                    coff = max(0, j) * P
                    s_ps = ps.tile([P, 2, 512], F32, tag="mm")
                    for h01 in range(2):
                        pb = h01 * HD
                        nc.tensor.matmul(
                            s_ps[:, h01, coff:512],
                            kT[pb : pb + HD, pair, kc * P : (kc + 1) * P],
                            qT[
                                pb : pb + HD,
                                pair,
                                qt * TB + coff : (qt + 1) * TB,
                            ],
                            start=True,
                            stop=True,
                        )
                    pT = pTp.tile([P, 2, 512], F16, tag="pT")
                    nc.scalar.activation(
                        pT[:, :, coff:512], s_ps[:, :, coff:512], Exp
                    )
                    if j >= 0:
                        nc.vector.tensor_tensor(
                            pT[:, :, coff : coff + P],
                            pT[:, :, coff : coff + P],
                            tri_h[:, None, :].to_broadcast((P, 2, P)),
                            MULT,
                        )
                    for h01 in range(2):
                        h = 2 * pair + h01
                        nc.tensor.matmul(
                            c_ps[0 : HD + 1, h01, coff:512],
                            v_heads[:, kc, h, :],
                            pT[:, h01, coff:512],
                            start=(kc == 0),
                            stop=(kc == nkc - 1),
                        )
                # stage raw ctx (+den placeholder) rows; collect den rows
                cxn = cxp.tile([HD + 1, 2, 512], F16, tag="cxn")
                nc.vector.tensor_copy(cxn[:], c_ps[0 : HD + 1, :, :])
                for sh in (qt, qt + 4):
                    nc.sync.dma_start(
                        a2a_in[pair][sh * SR : (sh + 1) * SR, :].rearrange(
                            "(h p) t -> p h t", h=2
                        )[0:HD],
                        cxn[0:HD],
                    )
                nc.sync.dma_start(
                    den_acc[pair][:]
                    .rearrange("(h q) t -> h q t", q=NQT)[:, qt : qt + 1, :],
                    cxn[HD : HD + 1, :, :],
                )

            def stage_rden(pair):
                """One [8, 512] reciprocal for the pair; rden rows into the
                A2A slots (overwrites the raw-den placeholder rows)."""
                rd32 = cxp.tile([2 * NQT, 512], F32, tag="rd32", name=f"rd32_{pair}")
                nc.vector.reciprocal(rd32[:], den_acc[pair][:])
                rdh = cxp.tile([2 * NQT, 512], F16, tag="rdh", name=f"rdh_{pair}")
                nc.vector.tensor_copy(rdh[:], rd32[:])
                a2a_r = a2a_in[pair][:].rearrange("(s r) t -> s r t", r=SR)
                for h01 in range(2):
                    dr = h01 * (HD + 1) + HD
                    for half in range(2):
                        nc.sync.dma_start(
                            a2a_r[4 * half : 4 * half + 4, dr : dr + 1, :],
                            rdh[h01 * NQT : (h01 + 1) * NQT, :],
                        )

            # phase 1: projections interleaved with pair-0 attention
            for g in range(NQT):
                if g > 0:
                    nc.sync.dma_start(
                        xTg[g][:], xt_r[:, :, g * TB : (g + 1) * TB]
                    )
                proj_group(g)
                attn(0, g)
            with tc.high_priority():
                stage_rden(0)
                if coll:
                    nc.gpsimd.collective_compute(
                        "AllToAll",
                        mybir.AluOpType.bypass,
                        ins=[a2a_in[0].opt()],
                        outs=[a2a_out[0].opt()],
                        replica_groups=[list(range(NC))],
                    )

            # weights/indices needed post-collective (overlap phase 2)
            nc.sync.dma_start(wo_sb[:], wo_d.rearrange("(dc p) n -> p dc n", p=P))
            bo_row = const.tile([1, D], F32, tag="bo_row")
            bo_bc = const.tile([P, D], F32, tag="bo_bc")
            nc.sync.dma_start(bo_row[:], bo2_d)
            nc.gpsimd.partition_broadcast(bo_bc[:], bo_row[:])
            gidx_sb = const.tile([P, D // 16], I16, tag="gidx")
            nc.sync.dma_start(gidx_sb[:], gidx_d)

            gsrc = a2a_out if coll else a2a_in
            outp = tc.alloc_tile_pool(name="outp", bufs=1)
            # cxr[:, 0:4] = raw ctx (-> normalized in place), [:, 4:8] = rden
            cxr = [
                outp.tile([P, 2 * NQT, TB], F16, tag=f"cxr{p}", name=f"cxr{p}")
                for p in range(2)
            ]

            # phase 2: pair-1 attention (overlaps AllToAll pair 0)
            for qt in range(NQT):
                attn(1, qt)
            with tc.high_priority():
                stage_rden(1)

            def recv_chain(pr):
                """Gather ctx + broadcast rden in one 1024-index gather, then
                normalize on gpsimd."""
                nc.gpsimd.dma_gather(
                    out_ap=cxr[pr][:],
                    in_ap=gsrc[pr][:],
                    idxs_ap=gidx_sb[:],
                    num_idxs=D,
                    num_idxs_reg=D,
                    elem_size=TB,
                )
                nc.vector.tensor_tensor(
                    cxr[pr][:, 0:NQT, :],
                    cxr[pr][:, 0:NQT, :],
                    cxr[pr][:, NQT : 2 * NQT, :],
                    MULT,
                )

            if coll:
                with tc.high_priority():
                    nc.gpsimd.collective_compute(
                        "AllToAll",
                        mybir.AluOpType.bypass,
                        ins=[a2a_in[1].opt()],
                        outs=[a2a_out[1].opt()],
                        replica_groups=[list(range(NC))],
                    )

            recv_chain(0)

            # out-proj even chunks (pair-0 dims): overlaps AllToAll pair 1
            with tc.tile_pool(name="out_pool", bufs=3) as out_pool:
                o_parts = [
                    outp.tile([P, 512], F32, tag=f"opart{u}", name=f"opart{u}")
                    for u in range(8)
                ]
                for u in range(8):
                    tc_i, nt = u // 2, u % 2
                    po = ps.tile([P, 2, 512], F32, tag="mm")
                    for i, g in enumerate(range(NQT)):
                        nc.tensor.matmul(
                            po[:, 0, :],
                            cxr[0][:, g, tc_i * P : (tc_i + 1) * P],
                            wo_sb[:, 2 * g, nt * 512 : (nt + 1) * 512],
                            start=(i == 0),
                            stop=(i == NQT - 1),
                        )
                    nc.vector.tensor_tensor(
                        o_parts[u][:],
                        po[:, 0, :],
                        bo_bc[:, nt * 512 : (nt + 1) * 512],
                        ADD,
                    )
                recv_chain(1)
                for u in range(8):
                    tc_i, nt = u // 2, u % 2
                    po = ps.tile([P, 2, 512], F32, tag="mm")
                    for i, g in enumerate(range(NQT)):
                        nc.tensor.matmul(
                            po[:, 0, :],
                            cxr[1][:, g, tc_i * P : (tc_i + 1) * P],
                            wo_sb[:, 2 * g + 1, nt * 512 : (nt + 1) * 512],
                            start=(i == 0),
                            stop=(i == NQT - 1),
                        )
                    o_sb = out_pool.tile([P, 512], F32, tag="osb")
                    nc.vector.tensor_tensor(
                        o_sb[:], po[:, 0, :], o_parts[u][:], ADD
                    )
                    nc.sync.dma_start(
                        y_d[
                            tc_i * P : (tc_i + 1) * P,
                            nt * 512 : (nt + 1) * 512,
                        ],
                        o_sb[:],
                    )

            outp.release()

    nc.compile()
    return nc


_NC_CACHE = {}


def _get_nc():
    if "nc" not in _NC_CACHE:
        _NC_CACHE["nc"] = build()
    return _NC_CACHE["nc"]


def _make_in_maps(x, Wq, bq, Wk, bk, Wv, bv, Wo, bo):
    x = np.asarray(x, np.float32)
    Wq, Wk, Wv, Wo = (np.asarray(a, np.float32) for a in (Wq, Wk, Wv, Wo))
    bq, bk, bv, bo = (np.asarray(a, np.float32) for a in (bq, bk, bv, bo))
    bo2 = (bv @ Wo + bo).astype(np.float32)  # fold bv into output bias
    in_maps = []
    for c in range(NC):
        b, g = c // 4, c % 4
        sl = slice(g * G, (g + 1) * G)
        # slot i of the gather = (partition i%128, free i//128); slot value
        # semantics: ctx dim (2g+pr)*128 + p comes from src head-group g,
        # head h01=p//64, row r64=p%64 at a2a row (b*4+g)*130 + h01*65 + r64.
        # idx consumption order: idx i read at [i%16, i//16] of the tile.
        i = np.arange(D // 2, dtype=np.int64)
        gsrc_, p_ = i // P, i % P
        h01_, r64_ = p_ // HD, p_ % HD
        ctx_v = (b * 4 + gsrc_) * SR + h01_ * (HD + 1) + r64_
        rd_v = (b * 4 + gsrc_) * SR + h01_ * (HD + 1) + HD
        both = np.concatenate([ctx_v, rd_v])  # 1024 idx values
        gidx = np.tile(
            both.reshape(D // 16, 16).T, (8, 1)
        ).astype(np.int16)
        in_maps.append(
            {
                "xt": np.ascontiguousarray(x[b].T).astype(np.float16),
                "wq": np.ascontiguousarray(Wq[:, sl] / 8.0).astype(np.float16),
                "wk": np.ascontiguousarray(Wk[:, sl]).astype(np.float16),
                "wv": np.ascontiguousarray(Wv[:, sl]).astype(np.float16),
                "bq": np.ascontiguousarray((bq[sl] / 8.0).reshape(MC, P).T),
                "wo": Wo.astype(np.float16),
                "bo2": np.ascontiguousarray(bo2.reshape(1, D)),
                "gidx": np.ascontiguousarray(gidx),
            }
        )
    return in_maps


def run(inputs, trace=False, tmpdir=None):
    """Run on 8 cores; returns (output [2,2048,1024], BassKernelResults)."""
    if trace:
        _install_ntff_hook()
    nc = _get_nc()
    in_maps = _make_in_maps(**inputs)
    res = bass_utils.run_bass_kernel_spmd(
        nc, in_maps, core_ids=list(range(NC)), trace=trace, tmpdir=tmpdir
    )
    out = np.empty((B, S, D), np.float32)
    for c in range(NC):
        b, g = c // 4, c % 4
        out[b, g * TB : (g + 1) * TB, :] = res.results[c]["y"]
    return out, res


def kernel(**inputs) -> np.ndarray:
    out, _ = run(inputs, trace=False)
    return out


# revision 17
# speedup vs baseline: 1.0651x; 1.0651x over previous
"""GPT self-attention layer (B=2, S=2048, D=1024, H=16, hd=64) on 8 TRN2 cores.

Sharding: data-parallel over batch (2) x tensor-parallel over heads (4 groups
of 4 heads). Core c handles batch b=c//4, head group g=c%4.

v3 (fp16, rden-in-A2A):
  - All PE operands fp16 (host-converted): fast weight load, no fp32r
    penalties, half DMA + collective bytes.
  - Host math folds: bk dropped (softmax shift invariance), bv folded into
    bo (bo2 = bv@Wo + bo), 1/sqrt(hd) folded into Wq/bq.
  - ACT runs ONLY Exp (single table load); Q/K psum evacuation on DVE;
    causal masks and receive-side normalize run on GpSimd so the DVE queue
    never blocks attention staging behind normalization work.
  - Softmax denominators: the ctx matmul's interleaved ones-column
    accumulates den into psum row 64.  Each (pair, qt) DMAs its den rows
    into den_acc; after a pair's last q-block ONE [8, 512] DVE reciprocal
    produces rden for the whole pair (a [1, 512] reciprocal costs 8 cyc/elem
    -- batching partitions is the only way it's cheap).  rden rows are
    staged into the A2A slots (130 rows per shard: 2 heads x (64 ctx + 1
    rden)).  The receiver runs ONE 1024-index gather per pair pulling ctx
    and broadcast rden together, then one gpsimd multiply normalizes.
  - A tiny warm-up AllToAll at kernel start absorbs CC stream setup.
"""

import contextlib
import ctypes
import sys
import types

sys.path.insert(0, "/opt/trn_rl_repo")

import numpy as np

import concourse.bass as bass
import concourse.mybir as mybir
import concourse.tile as tile
from concourse import bacc
from concourse import bass_utils

P = 128
B, S, D = 2, 2048, 1024
NH_LOC = 4          # heads per core
HD = 64             # head dim
G = NH_LOC * HD     # local head dims = 256
MC = G // P         # m-chunks of local dims = 2 (== head pairs)
DC = D // P         # d-chunks = 8
TB = 512            # token block (output tokens per core, q-tile width)
NQT = S // TB       # q-tiles = 4
NTC = S // P        # token chunks = 16
NC = 8
SR = 2 * (HD + 1)   # A2A slot rows per shard = 130

F32 = mybir.dt.float32
F16 = mybir.dt.float16
I16 = mybir.dt.int16
Exp = mybir.ActivationFunctionType.Exp
MULT = mybir.AluOpType.mult
ADD = mybir.AluOpType.add


def _install_ntff_hook():
    """Make trace=True work under axon: inject antenv.axon_hooks backed by
    ctypes calls into libaxon_pjrt.so (mirrors trn_agent_boot logic)."""
    if "antenv.axon_hooks" in sys.modules:
        return
    holder = {}
    mod = types.ModuleType("antenv.axon_hooks")
    mod.set_axon_ntff_profile_hook = lambda h: holder.update(h=h)
    mod.get_axon_ntff_profile_hook = lambda: holder.get("h")
    sys.modules["antenv.axon_hooks"] = mod
    try:
        lib = ctypes.CDLL("/opt/axon/libaxon_pjrt.so")
        if not hasattr(lib, "axon_start_nrt_profile"):
            return
    except OSError:
        return
    lib.axon_start_nrt_profile.argtypes = [
        ctypes.POINTER(ctypes.c_int64),
        ctypes.c_size_t,
    ]
    lib.axon_start_nrt_profile.restype = ctypes.c_int64
    lib.axon_stop_nrt_profile.argtypes = [ctypes.c_char_p]
    lib.axon_stop_nrt_profile.restype = ctypes.c_int64

    @contextlib.contextmanager
    def _hook(output_dir, device_ids):
        import jax

        jax.devices()
        if device_ids:
            ids = (ctypes.c_int64 * len(device_ids))(*device_ids)
            rc = lib.axon_start_nrt_profile(ids, len(device_ids))
        else:
            rc = lib.axon_start_nrt_profile(None, 0)
        if rc != 0:
            raise RuntimeError(f"axon_start_nrt_profile rc={rc}")
        try:
            yield
        finally:
            n = lib.axon_stop_nrt_profile(str(output_dir).encode())
            print(f"profile: {n} ntff file(s) written to {output_dir}")

    holder["h"] = _hook


def build(coll=True):
    nc = bacc.Bacc("TRN2", target_bir_lowering=False, debug=False, num_devices=NC)

    xt_d = nc.dram_tensor("xt", [D, S], F16, kind="ExternalInput").ap()
    wq_d = nc.dram_tensor("wq", [D, G], F16, kind="ExternalInput").ap()
    wk_d = nc.dram_tensor("wk", [D, G], F16, kind="ExternalInput").ap()
    wv_d = nc.dram_tensor("wv", [D, G], F16, kind="ExternalInput").ap()
    bq_d = nc.dram_tensor("bq", [P, MC], F32, kind="ExternalInput").ap()
    wo_d = nc.dram_tensor("wo", [D, D], F16, kind="ExternalInput").ap()
    bo2_d = nc.dram_tensor("bo2", [1, D], F32, kind="ExternalInput").ap()
    gidx_d = nc.dram_tensor("gidx", [P, D // 16], I16, kind="ExternalInput").ap()
    y_d = nc.dram_tensor("y", [TB, D], F32, kind="ExternalOutput").ap()

    with tile.TileContext(nc) as tc:
        with (
            tc.tile_pool(name="const", bufs=1) as const,
            tc.tile_pool(name="dram", bufs=1, space="DRAM") as dram,
            tc.tile_pool(name="ps", bufs=3, space="PSUM") as ps,
            tc.tile_pool(name="ps_ctx", bufs=1, space="PSUM") as ps_ctx,
            tc.tile_pool(name="persist", bufs=1) as persist,
            tc.tile_pool(name="pTp", bufs=8) as pTp,
            tc.tile_pool(name="cxp", bufs=4) as cxp,
        ):
            # warm-up collective: absorbs CC stream setup + initial barrier
            dummy_in = dram.tile([NC, 8], F16, tag="dummy_in")
            dummy_out = dram.tile([NC, 8], F16, tag="dummy_out")
            if coll:
                nc.gpsimd.collective_compute(
                    "AllToAll",
                    mybir.AluOpType.bypass,
                    ins=[dummy_in.opt()],
                    outs=[dummy_out.opt()],
                    replica_groups=[list(range(NC))],
                )

            # ---------------- weights/constants (DMA order matters) -------
            wq_sb = persist.tile([P, DC, G], F16, tag="wq")
            wk_sb = persist.tile([P, DC, G], F16, tag="wk")
            wv_sb = persist.tile([P, DC, G], F16, tag="wv")
            xTg = [
                persist.tile([P, DC, TB], F16, tag=f"xT{g}", name=f"xT{g}")
                for g in range(NQT)
            ]
            xt_r = xt_d.rearrange("(dc p) t -> p dc t", p=P)
            wq_r = wq_d.rearrange("(dc p) m -> p dc m", p=P)
            # interleave per-dc chunks so the first matmul starts early
            for dc in range(DC):
                nc.sync.dma_start(wq_sb[:, dc], wq_r[:, dc])
                nc.sync.dma_start(xTg[0][:, dc], xt_r[:, dc, 0:TB])
            bq_sb = const.tile([P, MC], F32, tag="bq")
            nc.sync.dma_start(bq_sb[:], bq_d)

            # trimask[k, u] = 1 if k <= u else 0 (keep where u - k >= 0)
            tri_f = const.tile([P, P], F32, tag="tri_f")
            nc.gpsimd.memset(tri_f[:], 1.0)
            nc.gpsimd.affine_select(
                out=tri_f[:],
                in_=tri_f[:],
                compare_op=mybir.AluOpType.is_ge,
                fill=0.0,
                base=0,
                pattern=[[1, P]],
                channel_multiplier=-1,
            )
            tri_h = const.tile([P, P], F16, tag="tri_h")
            nc.vector.tensor_copy(tri_h[:], tri_f[:])

            # persistent activations
            qT = persist.tile([P, MC, S], F16, tag="qT")
            kT = persist.tile([P, MC, S], F16, tag="kT")
            v_sb = persist.tile([P, NTC, NH_LOC * (HD + 1)], F16, tag="v")
            wo_sb = persist.tile([P, DC, D], F16, tag="wo")

            # ones columns of v (denominator trick): col 64 of each head block
            v_heads = v_sb[:].rearrange("p t (h c) -> p t h c", c=HD + 1)
            nc.vector.memset(v_heads[:, :, :, HD], 1.0)

            a2a_in = [
                dram.tile([NC * SR, TB], F16, name=f"a2ain{p}", tag=f"a2ain{p}")
                for p in range(2)
            ]
            a2a_out = [
                dram.tile([NC * SR, TB], F16, name=f"a2aout{p}", tag=f"a2aout{p}")
                for p in range(2)
            ]
            den_acc = [
                persist.tile([2 * NQT, TB], F16, tag=f"dacc{p}", name=f"dacc{p}")
                for p in range(2)
            ]

            def proj_group(g):
                """QT/KT/V projections for 512-token group g."""
                if g == 0:
                    nc.sync.dma_start(
                        wk_sb[:], wk_d.rearrange("(dc p) m -> p dc m", p=P)
                    )
                    nc.sync.dma_start(
                        wv_sb[:], wv_d.rearrange("(dc p) m -> p dc m", p=P)
                    )
                for w_sb, has_b, out_t in ((wq_sb, True, qT), (wk_sb, False, kT)):
                    for mc_i in range(MC):
                        pj = ps.tile([P, 2, 512], F32, tag="mm")
                        for dc in range(DC):
                            nc.tensor.matmul(
                                pj[:, 0, :],
                                w_sb[:, dc, mc_i * P : (mc_i + 1) * P],
                                xTg[g][:, dc, :],
                                start=(dc == 0),
                                stop=(dc == DC - 1),
                            )
                        dst = out_t[:, mc_i, g * TB : (g + 1) * TB]
                        if has_b:
                            nc.vector.tensor_scalar_add(
                                dst, pj[:, 0, :], bq_sb[:, mc_i : mc_i + 1]
                            )
                        else:
                            nc.vector.tensor_copy(dst, pj[:, 0, :])
                for ti in range(4):
                    tc_i = 4 * g + ti
                    pv = ps.tile([P, 2, 512], F32, tag="mm")
                    for dc in range(DC):
                        nc.tensor.matmul(
                            pv[:, 0, 0:G],
                            xTg[g][:, dc, ti * P : (ti + 1) * P],
                            wv_sb[:, dc, :],
                            start=(dc == 0),
                            stop=(dc == DC - 1),
                        )
                    nc.vector.tensor_copy(
                        v_heads[:, tc_i, :, 0:HD],
                        pv[:, 0, 0:G].rearrange("p (h c) -> p h c", c=HD),
                    )

            def attn(pair, qt):
                """Attention for head pair `pair`, q-block `qt`; stages raw
                ctx rows into a2a_in[pair] and den rows into den_acc."""
                nkc = 4 * qt + 4
                c_ps = ps_ctx.tile([P, 2, 512], F32, tag="ctx")
                for kc in range(nkc):
                    j = kc - 4 * qt
                    coff = max(0, j) * P
                    s_ps = ps.tile([P, 2, 512], F32, tag="mm")
                    for h01 in range(2):
                        pb = h01 * HD
                        nc.tensor.matmul(
                            s_ps[:, h01, coff:512],
                            kT[pb : pb + HD, pair, kc * P : (kc + 1) * P],
                            qT[
                                pb : pb + HD,
                                pair,
                                qt * TB + coff : (qt + 1) * TB,
                            ],
                            start=True,
                            stop=True,
                        )
                    pT = pTp.tile([P, 2, 512], F16, tag="pT")
                    nc.scalar.activation(
                        pT[:, :, coff:512], s_ps[:, :, coff:512], Exp
                    )
                    if j >= 0:
                        nc.vector.tensor_tensor(
                            pT[:, :, coff : coff + P],
                            pT[:, :, coff : coff + P],
                            tri_h[:, None, :].to_broadcast((P, 2, P)),
                            MULT,
                        )
                    for h01 in range(2):
                        h = 2 * pair + h01
                        nc.tensor.matmul(
                            c_ps[0 : HD + 1, h01, coff:512],
                            v_heads[:, kc, h, :],
                            pT[:, h01, coff:512],
                            start=(kc == 0),
                            stop=(kc == nkc - 1),
                        )
                # stage raw ctx rows; collect den rows (den DMA first so the
                # pair's reciprocal becomes ready as early as possible)
                cxn = cxp.tile([HD + 1, 2, 512], F16, tag="cxn")
                nc.vector.tensor_copy(cxn[:], c_ps[0 : HD + 1, :, :])
                nc.sync.dma_start(
                    den_acc[pair][:]
                    .rearrange("(h q) t -> h q t", q=NQT)[:, qt : qt + 1, :],
                    cxn[HD : HD + 1, :, :],
                )
                for sh in (qt, qt + 4):
                    nc.sync.dma_start(
                        a2a_in[pair][sh * SR : (sh + 1) * SR, :].rearrange(
                            "(h p) t -> p h t", h=2
                        )[0:HD],
                        cxn[0:HD],
                    )

            def stage_rden(pair):
                """One [8, 512] reciprocal for the pair; rden rows into the
                A2A slots (overwrites the raw-den placeholder rows)."""
                rd32 = cxp.tile([2 * NQT, 512], F32, tag="rd32", name=f"rd32_{pair}")
                nc.vector.reciprocal(rd32[:], den_acc[pair][:])
                rdh = cxp.tile([2 * NQT, 512], F16, tag="rdh", name=f"rdh_{pair}")
                nc.vector.tensor_copy(rdh[:], rd32[:])
                a2a_r = a2a_in[pair][:].rearrange("(s r) t -> s r t", r=SR)
                for h01 in range(2):
                    dr = h01 * (HD + 1) + HD
                    for half in range(2):
                        nc.sync.dma_start(
                            a2a_r[4 * half : 4 * half + 4, dr : dr + 1, :],
                            rdh[h01 * NQT : (h01 + 1) * NQT, :],
                        )

            # phase 1: projections interleaved with pair-0 attention
            for g in range(NQT):
                if g > 0:
                    nc.sync.dma_start(
                        xTg[g][:], xt_r[:, :, g * TB : (g + 1) * TB]
                    )
                proj_group(g)
                attn(0, g)
            with tc.high_priority():
                stage_rden(0)
                if coll:
                    nc.gpsimd.collective_compute(
                        "AllToAll",
                        mybir.AluOpType.bypass,
                        ins=[a2a_in[0].opt()],
                        outs=[a2a_out[0].opt()],
                        replica_groups=[list(range(NC))],
                    )

            # weights/indices needed post-collective (overlap phase 2)
            nc.sync.dma_start(wo_sb[:], wo_d.rearrange("(dc p) n -> p dc n", p=P))
            bo_row = const.tile([1, D], F32, tag="bo_row")
            bo_bc = const.tile([P, D], F32, tag="bo_bc")
            nc.sync.dma_start(bo_row[:], bo2_d)
            nc.gpsimd.partition_broadcast(bo_bc[:], bo_row[:])
            gidx_sb = const.tile([P, D // 16], I16, tag="gidx")
            nc.sync.dma_start(gidx_sb[:], gidx_d)

            gsrc = a2a_out if coll else a2a_in
            outp = tc.alloc_tile_pool(name="outp", bufs=1)
            # cxr[:, 0:4] = raw ctx (-> normalized in place), [:, 4:8] = rden
            cxr = [
                outp.tile([P, 2 * NQT, TB], F16, tag=f"cxr{p}", name=f"cxr{p}")
                for p in range(2)
            ]

            # phase 2: pair-1 attention (overlaps AllToAll pair 0)
            for qt in range(NQT):
                attn(1, qt)
            with tc.high_priority():
                stage_rden(1)

            def recv_chain(pr):
                """Gather ctx + broadcast rden in one 1024-index gather, then
                normalize on gpsimd."""
                nc.gpsimd.dma_gather(
                    out_ap=cxr[pr][:],
                    in_ap=gsrc[pr][:],
                    idxs_ap=gidx_sb[:],
                    num_idxs=D,
                    num_idxs_reg=D,
                    elem_size=TB,
                )
                nc.vector.tensor_tensor(
                    cxr[pr][:, 0:NQT, :],
                    cxr[pr][:, 0:NQT, :],
                    cxr[pr][:, NQT : 2 * NQT, :],
                    MULT,
                )

            if coll:
                with tc.high_priority():
                    nc.gpsimd.collective_compute(
                        "AllToAll",
                        mybir.AluOpType.bypass,
                        ins=[a2a_in[1].opt()],
                        outs=[a2a_out[1].opt()],
                        replica_groups=[list(range(NC))],
                    )

            # out-proj even chunks (pair-0 dims): overlaps AllToAll pair 1.
            # Deprioritized so the pair-1 rden/trigger chain wins scheduler
            # ties on the DVE/GpSimd queues.
            with tc.tile_pool(name="out_pool", bufs=3) as out_pool, \
                    tc.high_priority(offset=-1_000_000):
                recv_chain(0)
                o_parts = [
                    outp.tile([P, 512], F32, tag=f"opart{u}", name=f"opart{u}")
                    for u in range(8)
                ]
                for u in range(8):
                    tc_i, nt = u // 2, u % 2
                    po = ps.tile([P, 2, 512], F32, tag="mm")
                    for i, g in enumerate(range(NQT)):
                        nc.tensor.matmul(
                            po[:, 0, :],
                            cxr[0][:, g, tc_i * P : (tc_i + 1) * P],
                            wo_sb[:, 2 * g, nt * 512 : (nt + 1) * 512],
                            start=(i == 0),
                            stop=(i == NQT - 1),
                        )
                    nc.vector.tensor_tensor(
                        o_parts[u][:],
                        po[:, 0, :],
                        bo_bc[:, nt * 512 : (nt + 1) * 512],
                        ADD,
                    )
                recv_chain(1)
                for u in range(8):
                    tc_i, nt = u // 2, u % 2
                    po = ps.tile([P, 2, 512], F32, tag="mm")
                    for i, g in enumerate(range(NQT)):
                        nc.tensor.matmul(
                            po[:, 0, :],
                            cxr[1][:, g, tc_i * P : (tc_i + 1) * P],
                            wo_sb[:, 2 * g + 1, nt * 512 : (nt + 1) * 512],
                            start=(i == 0),
                            stop=(i == NQT - 1),
                        )
                    o_sb = out_pool.tile([P, 512], F32, tag="osb")
                    nc.vector.tensor_tensor(
                        o_sb[:], po[:, 0, :], o_parts[u][:], ADD
                    )
                    nc.sync.dma_start(
                        y_d[
                            tc_i * P : (tc_i + 1) * P,
                            nt * 512 : (nt + 1) * 512,
                        ],
                        o_sb[:],
                    )

            outp.release()

    nc.compile()
    return nc


_NC_CACHE = {}


def _get_nc():
    if "nc" not in _NC_CACHE:
        _NC_CACHE["nc"] = build()
    return _NC_CACHE["nc"]


def _make_in_maps(x, Wq, bq, Wk, bk, Wv, bv, Wo, bo):
    x = np.asarray(x, np.float32)
    Wq, Wk, Wv, Wo = (np.asarray(a, np.float32) for a in (Wq, Wk, Wv, Wo))
    bq, bk, bv, bo = (np.asarray(a, np.float32) for a in (bq, bk, bv, bo))
    bo2 = (bv @ Wo + bo).astype(np.float32)  # fold bv into output bias
    in_maps = []
    for c in range(NC):
        b, g = c // 4, c % 4
        sl = slice(g * G, (g + 1) * G)
        # slot i of the gather = (partition i%128, free i//128); slot value
        # semantics: ctx dim (2g+pr)*128 + p comes from src head-group g,
        # head h01=p//64, row r64=p%64 at a2a row (b*4+g)*130 + h01*65 + r64.
        # idx consumption order: idx i read at [i%16, i//16] of the tile.
        i = np.arange(D // 2, dtype=np.int64)
        gsrc_, p_ = i // P, i % P
        h01_, r64_ = p_ // HD, p_ % HD
        ctx_v = (b * 4 + gsrc_) * SR + h01_ * (HD + 1) + r64_
        rd_v = (b * 4 + gsrc_) * SR + h01_ * (HD + 1) + HD
        both = np.concatenate([ctx_v, rd_v])  # 1024 idx values
        gidx = np.tile(
            both.reshape(D // 16, 16).T, (8, 1)
        ).astype(np.int16)
        in_maps.append(
            {
                "xt": np.ascontiguousarray(x[b].T).astype(np.float16),
                "wq": np.ascontiguousarray(Wq[:, sl] / 8.0).astype(np.float16),
                "wk": np.ascontiguousarray(Wk[:, sl]).astype(np.float16),
                "wv": np.ascontiguousarray(Wv[:, sl]).astype(np.float16),
                "bq": np.ascontiguousarray((bq[sl] / 8.0).reshape(MC, P).T),
                "wo": Wo.astype(np.float16),
                "bo2": np.ascontiguousarray(bo2.reshape(1, D)),
                "gidx": np.ascontiguousarray(gidx),
            }
        )
    return in_maps


def run(inputs, trace=False, tmpdir=None):
    """Run on 8 cores; returns (output [2,2048,1024], BassKernelResults)."""
    if trace:
        _install_ntff_hook()
    nc = _get_nc()
    in_maps = _make_in_maps(**inputs)
    res = bass_utils.run_bass_kernel_spmd(
        nc, in_maps, core_ids=list(range(NC)), trace=trace, tmpdir=tmpdir
    )
    out = np.empty((B, S, D), np.float32)
    for c in range(NC):
        b, g = c // 4, c % 4
        out[b, g * TB : (g + 1) * TB, :] = res.results[c]["y"]
    return out, res


def kernel(**inputs) -> np.ndarray:
    out, _ = run(inputs, trace=False)
    return out
